# revision 6
# baseline (speedup 1.0000x reference)
"""Trainium2 Bass kernel for nn_Decoder_TRANSFORMER_14791867367496.

The reference decoder is affine in the positions: each frame step is
    pos_{t+1} = pos_t @ M + (d_t[b] + g[b,j]),   M = I + W_pe @ W3  (3x3)
(with W_final = [W1; W2; W3] split along its 768 input rows), so the whole
60-step scan has a closed form

    out[b, j, :, t] = X[b, j, :] @ Q_t + r_t[b, :]

where X = initial_grid,
    Q_t = M^t + (W_pe @ W2) @ S_t,          S_t = sum_{k<t} M^k
    r_t[b] = h @ S_t + D_t[b],              D_t = sum_{s=1..t} d_s M^{t-s}
    d_t[b] = (emb_table[t] + z @ W_clip + b_clip) @ W1
    h      = b_pe @ (W2 + W3) + b_final

All of Q/r are tiny (3x3 / per-batch 3-vectors) and are computed on the host
in float64.  The device kernel is then a single affine map per point
([3 feats + bias] -> 180 outputs).

Precision budget: the harness gate is rel_err < 2e-2.  The device output is
bf16 (rel RMS ~1.7e-3, measured) and upcast to f32 on gather, halving HBM
write traffic.  The matmuls run in fp8e5 (e5m2) with
MatmulPerfMode.DoubleRow, which streams 2 K-rows per PE cycle -> 150 ns per
[K=46]x[128,360] matmul instead of 300 ns for bf16.  e5m2 has 3 significand
bits, so each operand is split into chunks that sum to the f32 value:
X and Q into 3 chunks each with the 6 cross terms a+b<=2 kept (~2^-9
accuracy, bf16-parity), r (the large accumulated offset, |r| up to ~1.8e3 --
in e5m2 range, no scaling needed) into 5 chunks against 1.0 bias rows.
K-stack per tile = 6*3 + 5 = 23 rows; a pair of tiles is fused
block-diagonally per matmul (K_eff=46 = 23 partitions x 2 DoubleRow slots,
N=360).  Measured end-to-end scheme error: 1.65e-3.

Sharding: data-parallel over batch -- each of the 8 cores handles 4 batches
(16384 points = 128 point-tiles = 64 matmuls of 8 groups).  Critical path
on-device is the PSUM->SBUF drain: DVE and ACT each copy a multi-bank
strided slab per group (alternating 4/3 split to balance their 0.96/1.2 GHz
rates), converting f32 psum to bf16 stage lines that stream out in ~0.74 MB
linear DMAs.  All inputs arrive in one [23, 19264] fp8 tensor via two DMAs
(head = rhs + group-0 stationary so the first matmul starts ASAP).  The
first/last output groups go out as two half-DMAs to cut pipeline lead-in
and drain.
"""

import numpy as np

BS, NFRAMES, NJOINTS, NFEATS, LATENT, CLIP = 32, 60, 4096, 3, 256, 512
NCORES = 8
B_PER_CORE = BS // NCORES                  # 4
PTS = B_PER_CORE * NJOINTS                 # 16384 points per core
NTILES = PTS // 128                        # 128 point-tiles per core
GROUPS = 8                                 # output DMA groups
TPG = NTILES // GROUPS                     # 16 tiles per group
FC = NFEATS * NFRAMES                      # 180 output columns per point
KR = 23                                    # K rows per tile (18 cross + 5 r)
PAIR = 2                                   # tiles fused per matmul
MM_PER_G = TPG // PAIR                     # 8 matmuls per group
NMM = GROUPS * MM_PER_G                    # 64 matmuls
TERMS6 = [(0, 0), (0, 1), (1, 0), (0, 2), (1, 1), (2, 0)]  # (x_chunk, q_chunk)
RCH = 5                                    # e5m2 chunks for r

RHS_COLS = B_PER_CORE * 2 * PAIR * FC      # 4 batches x 720 = 2880 rhs cols
XT_PER_G = MM_PER_G * PAIR * 128           # 2048 input cols per group
IN_COLS = RHS_COLS + GROUPS * XT_PER_G     # 19264
HEAD_COLS = RHS_COLS + XT_PER_G            # rhs + group-0 stationary

N_STAGE = 3


def _split_f8(a, n):
    """Split f32 array into n e5m2 chunks whose sum approximates it to
    ~3n significand bits.  Returned as f32 arrays of e5m2-representable
    values."""
    import ml_dtypes
    f8 = ml_dtypes.float8_e5m2
    a = np.asarray(a, np.float32)
    out = []
    for _ in range(n):
        c = a.astype(f8).astype(np.float32)
        out.append(c)
        a = a - c
    return out


def _precompute(z, W_pe, b_pe, W_clip, b_clip, emb_table, W_final, b_final):
    """Host-side f64 computation of the closed-form coefficients.

    Returns Q_all [3, 180] and r_all [32, 180], column layout c = f*60 + t
    (matching the [.., 3, 60] innermost layout of the output)."""
    f64 = np.float64
    W_pe64 = np.asarray(W_pe, f64)
    W_fin = np.asarray(W_final, f64)
    W1, W2, W3 = W_fin[:LATENT], W_fin[LATENT:2 * LATENT], W_fin[2 * LATENT:]
    M = np.eye(3) + W_pe64 @ W3
    Gm = W_pe64 @ W2
    b_pe64 = np.asarray(b_pe, f64)
    h = b_pe64 @ W2 + b_pe64 @ W3 + np.asarray(b_final, f64)
    z_proj = np.asarray(z, f64) @ np.asarray(W_clip, f64) + np.asarray(b_clip, f64)
    d = (np.asarray(emb_table, f64)[None, :, :] + z_proj[:, None, :]) @ W1  # [32,60,3]

    Q = np.zeros((NFRAMES, 3, 3))
    R = np.zeros((NFRAMES, BS, 3))
    Q[0] = np.eye(3)
    Mt = np.eye(3)
    S = np.zeros((3, 3))
    D = np.zeros((BS, 3))
    for t in range(1, NFRAMES):
        S = S + Mt
        Mt = Mt @ M
        D = D @ M + d[:, t, :]
        Q[t] = Mt + Gm @ S
        R[t] = h @ S + D
    Q_all = Q.transpose(1, 2, 0).reshape(3, FC)     # [k, f*60+t]
    r_all = R.transpose(1, 2, 0).reshape(BS, FC)    # [b, f*60+t]
    return Q_all.astype(np.float32), r_all.astype(np.float32)


def _cv(g):
    """Slots copied by DVE in group g (ACT takes the rest of the 8).
    Alternating 4/3 matches the 0.96/1.2 GHz DVE/ACT rates on average."""
    return 4 if g % 2 == 0 else 3


def _build_bass():
    import concourse.mybir as mybir
    from concourse import bacc
    from concourse.bass import ts

    f32 = mybir.dt.float32
    bf16 = mybir.dt.bfloat16
    f8 = mybir.dt.float8e5
    DR = mybir.MatmulPerfMode.DoubleRow

    nc = bacc.Bacc(None, target_bir_lowering=False)
    inp = nc.dram_tensor("inp", [KR, IN_COLS], f8, kind="ExternalInput")
    out = nc.dram_tensor("out", [PTS, FC], bf16, kind="ExternalOutput")
    out_v = out[:].rearrange("(g j w) c -> g j (w c)", g=GROUPS, j=128, w=TPG)
    out_v4 = out[:].rearrange("(g j w) c -> g j w c", g=GROUPS, j=128, w=TPG)

    from contextlib import ExitStack
    ctx = ExitStack()
    in_sb = ctx.enter_context(nc.sbuf_tensor("in_sb", [KR, IN_COLS], f8))
    stage = [ctx.enter_context(
        nc.sbuf_tensor(f"stage{i}", [128, TPG * FC], bf16))
        for i in range(N_STAGE)]
    ps = ctx.enter_context(nc.psum_tensor("ps", [128, MM_PER_G, 512], f32))
    s_head = ctx.enter_context(nc.semaphore("s_head"))
    s_tail = ctx.enter_context(nc.semaphore("s_tail"))
    s_pe = ctx.enter_context(nc.semaphore("s_pe"))
    s_cpv = ctx.enter_context(nc.semaphore("s_cpv"))
    s_cpa = ctx.enter_context(nc.semaphore("s_cpa"))
    s_slot = [ctx.enter_context(nc.semaphore(f"s_slot{i}"))
              for i in range(N_STAGE)]

    def xt_ap(g, sp):
        base = RHS_COLS + XT_PER_G * g + PAIR * 128 * sp
        return in_sb[:, base:base + PAIR * 128].rearrange(
            "k (i m) -> k i m", i=2)

    def rhs_ap(lb):
        w = 2 * PAIR * FC
        return in_sb[:, w * lb:w * (lb + 1)].rearrange(
            "k (i n) -> k i n", i=2)

    # ---- input DMAs (both on the SP ring; head carries rhs + group-0
    # stationary so matmul 0 can start right after it lands) ----
    nc.sync.dma_start(out=in_sb[:, :HEAD_COLS],
                      in_=inp[:, :HEAD_COLS]).then_inc(s_head, 16)
    nc.sync.dma_start(out=in_sb[:, HEAD_COLS:],
                      in_=inp[:, HEAD_COLS:]).then_inc(s_tail, 16)

    # stage-slot reuse bookkeeping: how many out-DMA completions (x16) must
    # have hit s_slot[g%3] before group g may overwrite its stage
    dma_count = {0: 2, GROUPS - 1: 2}
    slot_reads_before = {}
    seen = [0] * N_STAGE
    for g in range(GROUPS):
        slot_reads_before[g] = seen[g % N_STAGE]
        seen[g % N_STAGE] += dma_count.get(g, 1)

    # ---- PE: one DoubleRow fp8 matmul per tile-pair ----
    for g in range(GROUPS):
        lb = g // 2
        for sp in range(MM_PER_G):
            j = g * MM_PER_G + sp
            if j == 0:
                nc.tensor.wait_ge(s_head, 16)
            elif j == MM_PER_G:
                nc.tensor.wait_ge(s_tail, 16)
            if j >= MM_PER_G:
                # psum slot sp was drained by group g-1's DVE or ACT copy
                if sp < _cv(g - 1):
                    nc.tensor.wait_ge(s_cpv, g)
                else:
                    nc.tensor.wait_ge(s_cpa, g)
            nc.tensor.matmul(
                ps[:, sp, 0:PAIR * FC],
                xt_ap(g, sp),
                rhs_ap(lb),
                start=True, stop=True,
                perf_mode=DR,
            ).then_inc(s_pe, 1)

    # ---- DVE: low psum slots of each group, one strided slab copy ----
    for g in range(GROUPS):
        cv = _cv(g)
        if g >= N_STAGE:
            nc.vector.wait_ge(s_slot[g % N_STAGE], 16 * slot_reads_before[g])
        nc.vector.wait_ge(s_pe, g * MM_PER_G + cv)
        nc.vector.tensor_copy(
            out=stage[g % N_STAGE][:, :cv * PAIR * FC].rearrange(
                "p (s c) -> p s c", c=PAIR * FC),
            in_=ps[:, 0:cv, 0:PAIR * FC],
        ).then_inc(s_cpv, 1)

    # ---- ACT: high psum slots of each group ----
    for g in range(GROUPS):
        cv = _cv(g)
        if g >= N_STAGE:
            nc.scalar.wait_ge(s_slot[g % N_STAGE], 16 * slot_reads_before[g])
        nc.scalar.wait_ge(s_pe, (g + 1) * MM_PER_G)
        nc.scalar.copy(
            out=stage[g % N_STAGE][:, cv * PAIR * FC:].rearrange(
                "p (s c) -> p s c", c=PAIR * FC),
            in_=ps[:, cv:MM_PER_G, 0:PAIR * FC],
        ).then_inc(s_cpa, 1)

    # ---- SP: output DMAs (first and last groups split in two so the
    # stream starts right after group 0's DVE slab and the tail drain is
    # half-length) ----
    for g in range(GROUPS):
        st = stage[g % N_STAGE]
        cv = _cv(g)
        if g == 0 or g == GROUPS - 1:
            w_mid = 2 * cv
            nc.sync.wait_ge(s_cpv, g + 1)
            nc.sync.dma_start(
                out=out_v4[g][:, 0:w_mid, :],
                in_=st[:, :w_mid * FC],
            ).then_inc(s_slot[g % N_STAGE], 16)
            nc.sync.wait_ge(s_cpa, g + 1)
            nc.sync.dma_start(
                out=out_v4[g][:, w_mid:TPG, :],
                in_=st[:, w_mid * FC:],
            ).then_inc(s_slot[g % N_STAGE], 16)
        else:
            nc.sync.wait_ge(s_cpv, g + 1)
            nc.sync.wait_ge(s_cpa, g + 1)
            nc.sync.dma_start(out=out_v[g], in_=st[:]).then_inc(
                s_slot[g % N_STAGE], 16)

    ctx.close()
    nc.finalize()
    return nc


_NC_CACHE = None
_LAST_RESULTS = None  # BassKernelResults of the most recent run (for profiling)


def kernel(z, mask, initial_grid, W_pe, b_pe, W_clip, b_clip, emb_table,
           W_final, b_final):
    global _NC_CACHE, _LAST_RESULTS
    import ml_dtypes
    from concourse import bass_utils

    f8 = ml_dtypes.float8_e5m2
    Q_all, r_all = _precompute(z, W_pe, b_pe, W_clip, b_clip, emb_table,
                               W_final, b_final)
    qc = _split_f8(Q_all, 3)                            # 3 x [3, 180]
    X = np.ascontiguousarray(np.asarray(initial_grid), dtype=np.float32)

    in_maps = []
    for c in range(NCORES):
        Xc = X[B_PER_CORE * c:B_PER_CORE * (c + 1)].reshape(PTS, NFEATS)
        # point p = g*2048 + j*16 + w lives at tile (g, w), psum partition j
        X4 = Xc.reshape(GROUPS, 128, TPG, NFEATS).transpose(3, 0, 2, 1)
        xch = _split_f8(X4, 3)                          # 3 x [3, 8, 16, 128]
        A = np.empty((GROUPS, TPG, KR, 128), np.float32)
        for k in range(NFEATS):
            for m, (a, b) in enumerate(TERMS6):
                A[:, :, 6 * k + m, :] = xch[a][k]
        A[:, :, 6 * NFEATS:KR, :] = 1.0                 # r bias rows
        # matmul sp of group g fuses tiles (2sp, 2sp+1); logical K-row
        # r=2p+i lands in partition p, DoubleRow slot i
        xt_host = (A.reshape(GROUPS, MM_PER_G, PAIR, KR, 128)
                   .reshape(GROUPS, MM_PER_G, PAIR * KR // 2, 2, 128)
                   .transpose(2, 0, 1, 3, 4)
                   .reshape(KR, GROUPS * XT_PER_G))

        rhs_host = np.zeros((KR, RHS_COLS), np.float32)
        for lb in range(B_PER_CORE):
            rs = _split_f8(r_all[B_PER_CORE * c + lb], RCH)  # RCH x [180]
            R = np.empty((KR, FC), np.float32)
            for k in range(NFEATS):
                for m, (a, b) in enumerate(TERMS6):
                    R[6 * k + m] = qc[b][k]
            R[6 * NFEATS:KR] = np.stack(rs)
            R46 = np.zeros((PAIR * KR, PAIR * FC), np.float32)
            for a in range(PAIR):                       # block-diagonal
                R46[KR * a:KR * (a + 1), FC * a:FC * (a + 1)] = R
            w = 2 * PAIR * FC
            rhs_host[:, w * lb:w * (lb + 1)] = R46.reshape(KR, w)
        inp_host = np.concatenate([rhs_host, xt_host], axis=1).astype(f8)
        in_maps.append({"inp": np.ascontiguousarray(inp_host)})

    if _NC_CACHE is None:
        _NC_CACHE = _build_bass()
    res = bass_utils.run_bass_kernel_spmd(
        _NC_CACHE, in_maps, core_ids=list(range(NCORES))
    )
    _LAST_RESULTS = res

    out = np.empty((BS, NJOINTS, NFEATS, NFRAMES), np.float32)
    for c in range(NCORES):
        out[B_PER_CORE * c:B_PER_CORE * (c + 1)] = (
            np.asarray(res.results[c]["out"]).astype(np.float32)
            .reshape(B_PER_CORE, NJOINTS, NFEATS, NFRAMES)
        )
    return out


# revision 8
# speedup vs baseline: 1.3955x; 1.3955x over previous
"""Trainium2 Bass kernel for nn_Decoder_TRANSFORMER_14791867367496.

The reference decoder is affine in the positions: each frame step is
    pos_{t+1} = pos_t @ M + (d_t[b] + g[b,j]),   M = I + W_pe @ W3  (3x3)
(with W_final = [W1; W2; W3] split along its 768 input rows), so the whole
60-step scan has a closed form

    out[b, j, :, t] = X[b, j, :] @ Q_t + r_t[b, :]

where X = initial_grid,
    Q_t = M^t + (W_pe @ W2) @ S_t,          S_t = sum_{k<t} M^k
    r_t[b] = h @ S_t + D_t[b],              D_t = sum_{s=1..t} d_s M^{t-s}
    d_t[b] = (emb_table[t] + z @ W_clip + b_clip) @ W1
    h      = b_pe @ (W2 + W3) + b_final

All of Q/r are tiny (3x3 / per-batch 3-vectors) and are computed on the host
in float64.  The device kernel is then a single affine map per point
([3 feats + bias] -> 180 outputs).

Precision budget: the harness gate is rel_err < 2e-2, so inputs are plain
bf16 (X and Q single-chunk; the large accumulated offset r split into two
bf16 chunks against two 1.0 bias rows) and the output is written as bf16
then upcast to f32 on gather.  Measured end-to-end error 1.7e-3.  K-stack
per tile is 5 rows, two tiles fused block-diagonally per matmul (K=10,
N=360).

The machine model this kernel is built around (all measured via NTFF):
PE streams one 128-wide psum column per cycle at 1.2 GHz regardless of
input dtype, so the 64 matmuls x 360 columns are a hard ~19.3 us floor;
DVE/ACT drain psum at 1 elem/lane/cycle (518/595 ns per [128,360] copy),
which exactly keeps pace with PE when alternated per matmul; each
dma_start's descriptors serialize on one of 16 queues (~200 GB/s alone,
~26 GB/s for DRAM->SBUF partition descriptors).  Hence:
 - per-matmul alternating DVE/ACT copies (never lets any engine block
   long enough to event-sleep),
 - input chunks spread over the SP, ACT and DVE rings so the first
   matmul's operands land as early as possible,
 - a dozen warm-up matmuls on garbage data keep PE busy until the input
   semaphore is already set (and give the p-state ramp a head start),
 - the first output group goes out in eighth/eighth/quarter/half pieces
   so the stream starts immediately, and the last group is split into
   four column-chunks on four queues to cut the ~3.6 us single-queue
   drain to ~1 us.

Sharding: data-parallel over batch -- each of the 8 cores handles 4
batches (16384 points = 128 point-tiles = 64 packed matmuls).
"""

import numpy as np

BS, NFRAMES, NJOINTS, NFEATS, LATENT, CLIP = 32, 60, 4096, 3, 256, 512
NCORES = 8
B_PER_CORE = BS // NCORES                  # 4
PTS = B_PER_CORE * NJOINTS                 # 16384 points per core
NTILES = PTS // 128                        # 128 point-tiles per core
GROUPS = 8                                 # output DMA groups
TPG = NTILES // GROUPS                     # 16 tiles per group
FC = NFEATS * NFRAMES                      # 180 output columns per point
KR = 5                                     # K rows per tile (3 feats + 2 bias)
PAIR = 2                                   # tiles fused per matmul
MM_PER_G = TPG // PAIR                     # 8 matmuls per group
N_WARM = 11                                # PE warm-up matmuls


def _split2(a):
    """Split f32 array into two bf16 chunks whose sum reproduces ~16
    mantissa bits.  Returned as f32 arrays holding bf16-representable
    values."""
    import ml_dtypes
    bf = ml_dtypes.bfloat16
    a = np.asarray(a, np.float32)
    a0 = a.astype(bf).astype(np.float32)
    a1 = (a - a0).astype(bf).astype(np.float32)
    return a0, a1


def _precompute(z, W_pe, b_pe, W_clip, b_clip, emb_table, W_final, b_final):
    """Host-side f64 computation of the closed-form coefficients.

    Returns Q_all [3, 180] and r_all [32, 180], column layout c = f*60 + t
    (matching the [.., 3, 60] innermost layout of the output)."""
    f64 = np.float64
    W_pe64 = np.asarray(W_pe, f64)
    W_fin = np.asarray(W_final, f64)
    W1, W2, W3 = W_fin[:LATENT], W_fin[LATENT:2 * LATENT], W_fin[2 * LATENT:]
    M = np.eye(3) + W_pe64 @ W3
    Gm = W_pe64 @ W2
    b_pe64 = np.asarray(b_pe, f64)
    h = b_pe64 @ W2 + b_pe64 @ W3 + np.asarray(b_final, f64)
    z_proj = np.asarray(z, f64) @ np.asarray(W_clip, f64) + np.asarray(b_clip, f64)
    d = (np.asarray(emb_table, f64)[None, :, :] + z_proj[:, None, :]) @ W1  # [32,60,3]

    Q = np.zeros((NFRAMES, 3, 3))
    R = np.zeros((NFRAMES, BS, 3))
    Q[0] = np.eye(3)
    Mt = np.eye(3)
    S = np.zeros((3, 3))
    D = np.zeros((BS, 3))
    for t in range(1, NFRAMES):
        S = S + Mt
        Mt = Mt @ M
        D = D @ M + d[:, t, :]
        Q[t] = Mt + Gm @ S
        R[t] = h @ S + D
    Q_all = Q.transpose(1, 2, 0).reshape(3, FC)     # [k, f*60+t]
    r_all = R.transpose(1, 2, 0).reshape(BS, FC)    # [b, f*60+t]
    return Q_all.astype(np.float32), r_all.astype(np.float32)


N_PS = 8      # psum slots (one bank each; a group cycles all 8)
N_STAGE = 3   # stage buffers


def _copy_seq(j):
    """(engine, 1-based position of copy j within that engine's stream).

    Copies alternate DVE/ACT by matmul index so both engines share every
    group's copy wall."""
    return ("v" if j % 2 == 0 else "a"), j // 2 + 1


def _build_bass():
    import concourse.mybir as mybir
    from concourse import bacc
    from concourse.bass import ts

    f32 = mybir.dt.float32
    bf16 = mybir.dt.bfloat16
    nc = bacc.Bacc(None, target_bir_lowering=False)
    xt = nc.dram_tensor("xt", [PAIR * KR, NTILES // PAIR * 128], bf16,
                        kind="ExternalInput")
    rhs = nc.dram_tensor("rhs", [PAIR * KR, B_PER_CORE * PAIR * FC], bf16,
                         kind="ExternalInput")
    out = nc.dram_tensor("out", [PTS, FC], bf16, kind="ExternalOutput")
    out_v = out[:].rearrange("(g j w) c -> g j (w c)", g=GROUPS, j=128, w=TPG)
    out_v4 = out[:].rearrange("(g j w) c -> g j w c", g=GROUPS, j=128, w=TPG)

    from contextlib import ExitStack
    ctx = ExitStack()
    rhs_sb = ctx.enter_context(
        nc.sbuf_tensor("rhs_sb", [PAIR * KR, B_PER_CORE * PAIR * FC], bf16))
    xt_sb = [ctx.enter_context(
        nc.sbuf_tensor(f"xt_sb{g}", [PAIR * KR, MM_PER_G * 128], bf16))
        for g in range(GROUPS)]
    stage = [ctx.enter_context(
        nc.sbuf_tensor(f"stage{i}", [128, TPG * FC], bf16))
        for i in range(N_STAGE)]
    psum = [ctx.enter_context(
        nc.psum_tensor(f"psum{i}", [128, PAIR * FC], f32))
        for i in range(N_PS)]
    s_rhs = ctx.enter_context(nc.semaphore("s_rhs"))
    s_c0a = ctx.enter_context(nc.semaphore("s_c0a"))
    s_chunk = [ctx.enter_context(nc.semaphore(f"s_chunk{g}"))
               for g in range(GROUPS)]
    s_pe = ctx.enter_context(nc.semaphore("s_pe"))
    s_cpv = ctx.enter_context(nc.semaphore("s_cpv"))
    s_cpa = ctx.enter_context(nc.semaphore("s_cpa"))
    s_slot = [ctx.enter_context(nc.semaphore(f"s_slot{i}"))
              for i in range(N_STAGE)]

    # ---- input DMAs, spread over three rings so the first matmul's
    # operands (chunk0a + rhs) land in parallel ASAP and later group
    # chunks trickle in behind.  Each dma_start's descriptors serialize
    # on one hw queue, so parallelism comes from separate dma_starts. ----
    half = MM_PER_G * 128 // 2
    nc.scalar.dma_start(out=xt_sb[0][:, :half],
                        in_=xt[:, :half]).then_inc(s_c0a, 16)
    nc.sync.dma_start(out=rhs_sb[:], in_=rhs[:]).then_inc(s_rhs, 16)
    nc.sync.dma_start(out=xt_sb[0][:, half:],
                      in_=xt[:, half:MM_PER_G * 128]).then_inc(s_chunk[0], 16)
    ring = {1: nc.scalar, 2: nc.scalar, 3: nc.sync, 4: nc.gpsimd,
            5: nc.gpsimd, 6: nc.gpsimd, 7: nc.gpsimd}
    for g in range(1, GROUPS):
        ring[g].dma_start(
            out=xt_sb[g][:], in_=xt[:, ts(g, MM_PER_G * 128)]
        ).then_inc(s_chunk[g], 16)

    # out-DMA inc totals per stage slot, recorded in emission order so the
    # slot-reuse waits below match however many DMAs read the slot.
    dma_count = {0: 4, GROUPS - 1: 4}
    slot_reads_before = {}
    seen = [0] * N_STAGE
    for g in range(GROUPS):
        slot_reads_before[g] = seen[g % N_STAGE]
        seen[g % N_STAGE] += dma_count.get(g, 1)

    def copies(engine, s_cp_self, g, parity):
        st = stage[g % N_STAGE]
        for sp in range(parity, MM_PER_G, 2):
            j = g * MM_PER_G + sp
            if sp == parity and g >= N_STAGE:
                # stage slot reuse: wait for every out-DMA that read it
                engine.wait_ge(s_slot[g % N_STAGE],
                               16 * slot_reads_before[g])
            engine.wait_ge(s_pe, j + 1)
            if parity == 0:
                nc.vector.tensor_copy(
                    out=st[:, ts(sp, PAIR * FC)], in_=psum[j % N_PS][:]
                ).then_inc(s_cp_self, 1)
            else:
                nc.scalar.copy(
                    out=st[:, ts(sp, PAIR * FC)], in_=psum[j % N_PS][:]
                ).then_inc(s_cp_self, 1)

    for g in range(GROUPS):
        copies(nc.scalar, s_cpa, g, 1)

    # ---- DVE: even-slot copies ----
    for g in range(GROUPS):
        copies(nc.vector, s_cpv, g, 0)

    # ---- PE: warm-up matmuls on garbage data (keep the engine busy and
    # the clock ramping until the input semaphores are already set), then
    # the real matmuls ----
    for w in range(N_WARM):
        nc.tensor.matmul(
            psum[N_PS - 1][:],
            xt_sb[GROUPS - 1][:, ts(w % MM_PER_G, 128)],
            rhs_sb[:, :PAIR * FC],
            start=True, stop=True,
        )
    for g in range(GROUPS):
        lb = g // 2
        for sp in range(MM_PER_G):
            j = g * MM_PER_G + sp
            if g == 0:
                if sp == 0:
                    nc.tensor.wait_ge(s_c0a, 16)
                    nc.tensor.wait_ge(s_rhs, 16)
                elif sp == MM_PER_G // 2:
                    nc.tensor.wait_ge(s_chunk[0], 16)
            elif sp == 0:
                nc.tensor.wait_ge(s_chunk[g], 16)
            if j >= N_PS:
                # psum slot reuse: wait for the copy that drained it
                eng, pos = _copy_seq(j - N_PS)
                nc.tensor.wait_ge(s_cpv if eng == "v" else s_cpa, pos)
            nc.tensor.matmul(
                psum[j % N_PS][:],
                xt_sb[g][:, ts(sp, 128)],
                rhs_sb[:, ts(lb, PAIR * FC)],
                start=True, stop=True,
            ).then_inc(s_pe, 1)

    # ---- SP: output DMAs ----
    for g in range(GROUPS):
        st = stage[g % N_STAGE]
        if g == 0:
            # eighth/eighth/quarter/half DMAs: the stream starts right
            # after matmul 0's copy lands
            for nv, na, w0, w1 in ((1, 0, 0, 2), (1, 1, 2, 4),
                                   (2, 2, 4, 8), (4, 4, 8, TPG)):
                nc.sync.wait_ge(s_cpv, nv)
                if na:
                    nc.sync.wait_ge(s_cpa, na)
                nc.sync.dma_start(
                    out=out_v4[0][:, w0:w1, :],
                    in_=st[:, w0 * FC:w1 * FC],
                ).then_inc(s_slot[0], 16)
            continue
        n_half = MM_PER_G * (g + 1) // 2
        if g == GROUPS - 1:
            # last group: four quarter DMAs on four queues so the final
            # drain is ~4x shorter than one serialized 0.74 MB DMA
            for q in range(4):
                w0, w1 = q * TPG // 4, (q + 1) * TPG // 4
                nv = na = MM_PER_G * g // 2 + (q + 1)
                nc.sync.wait_ge(s_cpv, nv)
                nc.sync.wait_ge(s_cpa, na)
                nc.sync.dma_start(
                    out=out_v4[g][:, w0:w1, :],
                    in_=st[:, w0 * FC:w1 * FC],
                ).then_inc(s_slot[g % N_STAGE], 16)
            continue
        nc.sync.wait_ge(s_cpv, n_half)
        nc.sync.wait_ge(s_cpa, n_half)
        nc.sync.dma_start(out=out_v[g], in_=st[:]).then_inc(
            s_slot[g % N_STAGE], 16)

    ctx.close()
    nc.finalize()
    return nc


_NC_CACHE = None
_LAST_RESULTS = None  # BassKernelResults of the most recent run (for profiling)


def kernel(z, mask, initial_grid, W_pe, b_pe, W_clip, b_clip, emb_table,
           W_final, b_final):
    global _NC_CACHE, _LAST_RESULTS
    import ml_dtypes
    from concourse import bass_utils

    bf = ml_dtypes.bfloat16
    Q_all, r_all = _precompute(z, W_pe, b_pe, W_clip, b_clip, emb_table,
                               W_final, b_final)
    Q0 = Q_all.astype(bf).astype(np.float32)            # [3, 180]
    X = np.ascontiguousarray(np.asarray(initial_grid), dtype=np.float32)

    in_maps = []
    for c in range(NCORES):
        Xc = X[B_PER_CORE * c:B_PER_CORE * (c + 1)].reshape(PTS, NFEATS)
        # point p = g*2048 + j*16 + w lives at tile (g, w), psum partition j
        X4 = Xc.reshape(GROUPS, 128, TPG, NFEATS).transpose(3, 0, 2, 1)
        A = np.empty((GROUPS, TPG, KR, 128), np.float32)
        A[:, :, 0:NFEATS, :] = X4.transpose(1, 2, 0, 3)  # single bf16 chunk
        A[:, :, NFEATS:KR, :] = 1.0                      # bias rows (r0, r1)
        # matmul s covers tiles (2*(s%8), 2*(s%8)+1) of group s//8;
        # stationary rows KR*a.. hold tile a of the pair
        xt_host = (A.reshape(GROUPS, MM_PER_G, PAIR, KR, 128)
                   .transpose(2, 3, 0, 1, 4)
                   .reshape(PAIR * KR, NTILES // PAIR * 128)).astype(bf)

        rhs_host = np.zeros((PAIR * KR, B_PER_CORE * PAIR * FC), np.float32)
        for lb in range(B_PER_CORE):
            r0, r1 = _split2(r_all[B_PER_CORE * c + lb])  # 2 x [180]
            R = np.empty((KR, FC), np.float32)
            R[0:NFEATS] = Q0
            R[NFEATS] = r0
            R[NFEATS + 1] = r1
            for a in range(PAIR):                       # block-diagonal
                rhs_host[KR * a:KR * (a + 1),
                         lb * PAIR * FC + FC * a: lb * PAIR * FC + FC * (a + 1)] = R
        in_maps.append({"xt": np.ascontiguousarray(xt_host),
                        "rhs": rhs_host.astype(bf)})

    if _NC_CACHE is None:
        _NC_CACHE = _build_bass()
    res = bass_utils.run_bass_kernel_spmd(
        _NC_CACHE, in_maps, core_ids=list(range(NCORES))
    )
    _LAST_RESULTS = res

    out = np.empty((BS, NJOINTS, NFEATS, NFRAMES), np.float32)
    for c in range(NCORES):
        out[B_PER_CORE * c:B_PER_CORE * (c + 1)] = (
            np.asarray(res.results[c]["out"]).astype(np.float32)
            .reshape(B_PER_CORE, NJOINTS, NFEATS, NFRAMES)
        )
    return out


# revision 12
# speedup vs baseline: 1.6826x; 1.2057x over previous
"""Trainium2 Bass kernel for nn_Decoder_TRANSFORMER_14791867367496.

The reference decoder is affine in the positions: each frame step is
    pos_{t+1} = pos_t @ M + (d_t[b] + g[b,j]),   M = I + W_pe @ W3  (3x3)
(with W_final = [W1; W2; W3] split along its 768 input rows), so the whole
60-step scan has a closed form

    out[b, j, :, t] = X[b, j, :] @ Q_t + r_t[b, :]

where X = initial_grid,
    Q_t = M^t + (W_pe @ W2) @ S_t,          S_t = sum_{k<t} M^k
    r_t[b] = h @ S_t + D_t[b],              D_t = sum_{s=1..t} d_s M^{t-s}
    d_t[b] = (emb_table[t] + z @ W_clip + b_clip) @ W1
    h      = b_pe @ (W2 + W3) + b_final

All of Q/r are tiny (3x3 / per-batch 3-vectors) and are computed on the host
in float64.  The device kernel is then a single affine map per point
([3 feats + bias] -> 180 outputs).

Precision budget: the harness gate is rel_err < 2e-2, so inputs are plain
bf16 (X and Q single-chunk; the large accumulated offset r split into two
bf16 chunks against two 1.0 bias rows) and the output is written as bf16
then upcast to f32 on gather.  Measured end-to-end error 1.7e-3.  K-stack
per tile is 5 rows, two tiles fused block-diagonally per matmul (K=10,
N=360).

The machine model this kernel is built around (all measured via NTFF):
PE streams one 128-wide psum column per cycle at 1.2 GHz regardless of
input dtype, so the 64 matmuls x 360 columns are a hard ~19.3 us floor;
DVE/ACT drain psum at 1 elem/lane/cycle (518/595 ns per [128,360] copy),
which exactly keeps pace with PE when alternated per matmul; each
dma_start's descriptors serialize on one of 16 queues (~200 GB/s alone,
~26 GB/s for DRAM->SBUF partition descriptors).  Hence:
 - per-matmul alternating DVE/ACT copies (never lets any engine block
   long enough to event-sleep),
 - input chunks spread over the SP, ACT and DVE rings so the first
   matmul's operands land as early as possible,
 - a dozen warm-up matmuls on garbage data keep PE busy until the input
   semaphore is already set (and give the p-state ramp a head start),
 - the first output group goes out in eighth/eighth/quarter/half pieces
   so the stream starts immediately, and the last group is split into
   four column-chunks on four queues to cut the ~3.6 us single-queue
   drain to ~1 us.

Sharding: data-parallel over batch -- each of the 8 cores handles 4
batches (16384 points = 128 point-tiles = 64 packed matmuls).
"""

import numpy as np

BS, NFRAMES, NJOINTS, NFEATS, LATENT, CLIP = 32, 60, 4096, 3, 256, 512
NCORES = 8
B_PER_CORE = BS // NCORES                  # 4
PTS = B_PER_CORE * NJOINTS                 # 16384 points per core
NTILES = PTS // 128                        # 128 point-tiles per core
GROUPS = 8                                 # output DMA groups
TPG = NTILES // GROUPS                     # 16 tiles per group
FC = NFEATS * NFRAMES                      # 180 output columns per point
KR = 5                                     # K rows per tile (3 feats + 2 bias)
PAIR = 2                                   # tiles fused per matmul
MM_PER_G = TPG // PAIR                     # 8 matmuls per group
N_WARM = 9                                 # PE warm-up matmuls


def _split2(a):
    """Split f32 array into two bf16 chunks whose sum reproduces ~16
    mantissa bits.  Returned as f32 arrays holding bf16-representable
    values."""
    import ml_dtypes
    bf = ml_dtypes.bfloat16
    a = np.asarray(a, np.float32)
    a0 = a.astype(bf).astype(np.float32)
    a1 = (a - a0).astype(bf).astype(np.float32)
    return a0, a1


def _precompute(z, W_pe, b_pe, W_clip, b_clip, emb_table, W_final, b_final):
    """Host-side f64 computation of the closed-form coefficients.

    Returns Q_all [3, 180] and r_all [32, 180], column layout c = f*60 + t
    (matching the [.., 3, 60] innermost layout of the output)."""
    f64 = np.float64
    W_pe64 = np.asarray(W_pe, f64)
    W_fin = np.asarray(W_final, f64)
    W1, W2, W3 = W_fin[:LATENT], W_fin[LATENT:2 * LATENT], W_fin[2 * LATENT:]
    M = np.eye(3) + W_pe64 @ W3
    Gm = W_pe64 @ W2
    b_pe64 = np.asarray(b_pe, f64)
    h = b_pe64 @ W2 + b_pe64 @ W3 + np.asarray(b_final, f64)
    z_proj = np.asarray(z, f64) @ np.asarray(W_clip, f64) + np.asarray(b_clip, f64)
    d = (np.asarray(emb_table, f64)[None, :, :] + z_proj[:, None, :]) @ W1  # [32,60,3]

    Q = np.zeros((NFRAMES, 3, 3))
    R = np.zeros((NFRAMES, BS, 3))
    Q[0] = np.eye(3)
    Mt = np.eye(3)
    S = np.zeros((3, 3))
    D = np.zeros((BS, 3))
    for t in range(1, NFRAMES):
        S = S + Mt
        Mt = Mt @ M
        D = D @ M + d[:, t, :]
        Q[t] = Mt + Gm @ S
        R[t] = h @ S + D
    Q_all = Q.transpose(1, 2, 0).reshape(3, FC)     # [k, f*60+t]
    r_all = R.transpose(1, 2, 0).reshape(BS, FC)    # [b, f*60+t]
    return Q_all.astype(np.float32), r_all.astype(np.float32)


N_PS = 8      # psum slots (one bank each; a group cycles all 8)
N_STAGE = 3   # stage buffers


def _copy_seq(j):
    """(engine, 1-based position of copy j within that engine's stream).

    Copies alternate DVE/ACT by matmul index so both engines share every
    group's copy wall."""
    return ("v" if j % 2 == 0 else "a"), j // 2 + 1


def _build_bass():
    import concourse.mybir as mybir
    from concourse import bacc
    from concourse.bass import ts

    f32 = mybir.dt.float32
    bf16 = mybir.dt.bfloat16
    nc = bacc.Bacc(None, target_bir_lowering=False)
    xt = nc.dram_tensor("xt", [PAIR * KR, NTILES // PAIR * 128], bf16,
                        kind="ExternalInput")
    rhs = nc.dram_tensor("rhs", [PAIR * KR, B_PER_CORE * PAIR * FC], bf16,
                         kind="ExternalInput")
    out = nc.dram_tensor("out", [PTS, FC], bf16, kind="ExternalOutput")
    out_v = out[:].rearrange("(g j w) c -> g j (w c)", g=GROUPS, j=128, w=TPG)
    out_v4 = out[:].rearrange("(g j w) c -> g j w c", g=GROUPS, j=128, w=TPG)

    from contextlib import ExitStack
    ctx = ExitStack()
    rhs_sb = ctx.enter_context(
        nc.sbuf_tensor("rhs_sb", [PAIR * KR, B_PER_CORE * PAIR * FC], bf16))
    xt_sb = [ctx.enter_context(
        nc.sbuf_tensor(f"xt_sb{g}", [PAIR * KR, MM_PER_G * 128], bf16))
        for g in range(GROUPS)]
    stage = [ctx.enter_context(
        nc.sbuf_tensor(f"stage{i}", [128, TPG * FC], bf16))
        for i in range(N_STAGE)]
    psum = [ctx.enter_context(
        nc.psum_tensor(f"psum{i}", [128, PAIR * FC], f32))
        for i in range(N_PS)]
    s_rhs = ctx.enter_context(nc.semaphore("s_rhs"))
    s_c0a = ctx.enter_context(nc.semaphore("s_c0a"))
    s_chunk = [ctx.enter_context(nc.semaphore(f"s_chunk{g}"))
               for g in range(GROUPS)]
    s_pe = ctx.enter_context(nc.semaphore("s_pe"))
    s_cpv = ctx.enter_context(nc.semaphore("s_cpv"))
    s_cpa = ctx.enter_context(nc.semaphore("s_cpa"))
    s_slot = [ctx.enter_context(nc.semaphore(f"s_slot{i}"))
              for i in range(N_STAGE)]

    # ---- input DMAs, spread over three rings so the first matmul's
    # operands (chunk0a + rhs) land in parallel ASAP and later group
    # chunks trickle in behind.  Each dma_start's descriptors serialize
    # on one hw queue, so parallelism comes from separate dma_starts. ----
    half = MM_PER_G * 128 // 2
    nc.sync.dma_start(out=xt_sb[0][:, :half],
                      in_=xt[:, :half]).then_inc(s_c0a, 16)
    nc.gpsimd.dma_start(out=rhs_sb[:], in_=rhs[:]).then_inc(s_rhs, 16)
    nc.scalar.dma_start(out=xt_sb[0][:, half:],
                        in_=xt[:, half:MM_PER_G * 128]).then_inc(s_chunk[0], 16)
    ring = {1: nc.scalar, 2: nc.scalar, 3: nc.sync, 4: nc.gpsimd,
            5: nc.gpsimd, 6: nc.gpsimd, 7: nc.gpsimd}
    for g in range(1, GROUPS):
        ring[g].dma_start(
            out=xt_sb[g][:], in_=xt[:, ts(g, MM_PER_G * 128)]
        ).then_inc(s_chunk[g], 16)

    # out-DMA inc totals per stage slot, recorded in emission order so the
    # slot-reuse waits below match however many DMAs read the slot.
    dma_count = {0: 3, GROUPS - 2: 2, GROUPS - 1: 4}
    slot_reads_before = {}
    seen = [0] * N_STAGE
    for g in range(GROUPS):
        slot_reads_before[g] = seen[g % N_STAGE]
        seen[g % N_STAGE] += dma_count.get(g, 1)

    def copies(engine, s_cp_self, g, parity):
        st = stage[g % N_STAGE]
        for sp in range(parity, MM_PER_G, 2):
            j = g * MM_PER_G + sp
            if sp == parity and g >= N_STAGE:
                # stage slot reuse: wait for every out-DMA that read it
                engine.wait_ge(s_slot[g % N_STAGE],
                               16 * slot_reads_before[g])
            engine.wait_ge(s_pe, j + 1)
            if parity == 0:
                nc.vector.tensor_copy(
                    out=st[:, ts(sp, PAIR * FC)], in_=psum[j % N_PS][:]
                ).then_inc(s_cp_self, 1)
            else:
                nc.scalar.copy(
                    out=st[:, ts(sp, PAIR * FC)], in_=psum[j % N_PS][:]
                ).then_inc(s_cp_self, 1)

    for g in range(GROUPS):
        copies(nc.scalar, s_cpa, g, 1)

    # ---- DVE: even-slot copies ----
    for g in range(GROUPS):
        copies(nc.vector, s_cpv, g, 0)

    # ---- PE: warm-up matmuls on garbage data (keep the engine busy and
    # the clock ramping until the input semaphores are already set), then
    # the real matmuls ----
    for w in range(N_WARM):
        nc.tensor.matmul(
            psum[N_PS - 1][:],
            xt_sb[GROUPS - 1][:, ts(w % MM_PER_G, 128)],
            rhs_sb[:, :PAIR * FC],
            start=True, stop=True,
        )
    for g in range(GROUPS):
        lb = g // 2
        for sp in range(MM_PER_G):
            j = g * MM_PER_G + sp
            if g == 0:
                if sp == 0:
                    nc.tensor.wait_ge(s_c0a, 16)
                    nc.tensor.wait_ge(s_rhs, 16)
                elif sp == MM_PER_G // 2:
                    nc.tensor.wait_ge(s_chunk[0], 16)
            elif sp == 0:
                nc.tensor.wait_ge(s_chunk[g], 16)
            if j >= N_PS:
                # psum slot reuse: wait for the copy that drained it
                eng, pos = _copy_seq(j - N_PS)
                nc.tensor.wait_ge(s_cpv if eng == "v" else s_cpa, pos)
            nc.tensor.matmul(
                psum[j % N_PS][:],
                xt_sb[g][:, ts(sp, 128)],
                rhs_sb[:, ts(lb, PAIR * FC)],
                start=True, stop=True,
            ).then_inc(s_pe, 1)

    # ---- SP: output DMAs ----
    for g in range(GROUPS):
        st = stage[g % N_STAGE]
        if g == 0:
            # quarter/quarter/half DMAs: the stream starts right after the
            # first two copies land, with descriptor-efficient sizes
            for nv, na, w0, w1 in ((1, 1, 0, 4), (2, 2, 4, 8),
                                   (4, 4, 8, TPG)):
                nc.sync.wait_ge(s_cpv, nv)
                nc.sync.wait_ge(s_cpa, na)
                nc.sync.dma_start(
                    out=out_v4[0][:, w0:w1, :],
                    in_=st[:, w0 * FC:w1 * FC],
                ).then_inc(s_slot[0], 16)
            continue
        n_half = MM_PER_G * (g + 1) // 2
        if g == GROUPS - 2:
            # second-to-last group in halves to start its drain earlier
            for q in range(2):
                w0, w1 = q * TPG // 2, (q + 1) * TPG // 2
                n = MM_PER_G * g // 2 + 2 * (q + 1)
                nc.sync.wait_ge(s_cpv, n)
                nc.sync.wait_ge(s_cpa, n)
                nc.sync.dma_start(
                    out=out_v4[g][:, w0:w1, :],
                    in_=st[:, w0 * FC:w1 * FC],
                ).then_inc(s_slot[g % N_STAGE], 16)
            continue
        if g == GROUPS - 1:
            # last group: four quarter DMAs on four queues so the final
            # drain is ~4x shorter than one serialized 0.74 MB DMA
            for q in range(4):
                w0, w1 = q * TPG // 4, (q + 1) * TPG // 4
                nv = na = MM_PER_G * g // 2 + (q + 1)
                nc.sync.wait_ge(s_cpv, nv)
                nc.sync.wait_ge(s_cpa, na)
                nc.sync.dma_start(
                    out=out_v4[g][:, w0:w1, :],
                    in_=st[:, w0 * FC:w1 * FC],
                ).then_inc(s_slot[g % N_STAGE], 16)
            continue
        nc.sync.wait_ge(s_cpv, n_half)
        nc.sync.wait_ge(s_cpa, n_half)
        nc.sync.dma_start(out=out_v[g], in_=st[:]).then_inc(
            s_slot[g % N_STAGE], 16)

    ctx.close()
    nc.finalize()
    return nc


_NC_CACHE = None
_LAST_RESULTS = None  # BassKernelResults of the most recent run (for profiling)


def kernel(z, mask, initial_grid, W_pe, b_pe, W_clip, b_clip, emb_table,
           W_final, b_final):
    global _NC_CACHE, _LAST_RESULTS
    import ml_dtypes
    from concourse import bass_utils

    bf = ml_dtypes.bfloat16
    Q_all, r_all = _precompute(z, W_pe, b_pe, W_clip, b_clip, emb_table,
                               W_final, b_final)
    Q0 = Q_all.astype(bf).astype(np.float32)            # [3, 180]
    X = np.ascontiguousarray(np.asarray(initial_grid), dtype=np.float32)

    in_maps = []
    for c in range(NCORES):
        Xc = X[B_PER_CORE * c:B_PER_CORE * (c + 1)].reshape(PTS, NFEATS)
        # point p = g*2048 + j*16 + w lives at tile (g, w), psum partition j
        X4 = Xc.reshape(GROUPS, 128, TPG, NFEATS).transpose(3, 0, 2, 1)
        A = np.empty((GROUPS, TPG, KR, 128), np.float32)
        A[:, :, 0:NFEATS, :] = X4.transpose(1, 2, 0, 3)  # single bf16 chunk
        A[:, :, NFEATS:KR, :] = 1.0                      # bias rows (r0, r1)
        # matmul s covers tiles (2*(s%8), 2*(s%8)+1) of group s//8;
        # stationary rows KR*a.. hold tile a of the pair
        xt_host = (A.reshape(GROUPS, MM_PER_G, PAIR, KR, 128)
                   .transpose(2, 3, 0, 1, 4)
                   .reshape(PAIR * KR, NTILES // PAIR * 128)).astype(bf)

        rhs_host = np.zeros((PAIR * KR, B_PER_CORE * PAIR * FC), np.float32)
        for lb in range(B_PER_CORE):
            r0, r1 = _split2(r_all[B_PER_CORE * c + lb])  # 2 x [180]
            R = np.empty((KR, FC), np.float32)
            R[0:NFEATS] = Q0
            R[NFEATS] = r0
            R[NFEATS + 1] = r1
            for a in range(PAIR):                       # block-diagonal
                rhs_host[KR * a:KR * (a + 1),
                         lb * PAIR * FC + FC * a: lb * PAIR * FC + FC * (a + 1)] = R
        in_maps.append({"xt": np.ascontiguousarray(xt_host),
                        "rhs": rhs_host.astype(bf)})

    if _NC_CACHE is None:
        _NC_CACHE = _build_bass()
    res = bass_utils.run_bass_kernel_spmd(
        _NC_CACHE, in_maps, core_ids=list(range(NCORES))
    )
    _LAST_RESULTS = res

    out = np.empty((BS, NJOINTS, NFEATS, NFRAMES), np.float32)
    for c in range(NCORES):
        out[B_PER_CORE * c:B_PER_CORE * (c + 1)] = (
            np.asarray(res.results[c]["out"]).astype(np.float32)
            .reshape(B_PER_CORE, NJOINTS, NFEATS, NFRAMES)
        )
    return out


# revision 19
# speedup vs baseline: 1.7162x; 1.0200x over previous
"""Trainium2 Bass kernel for nn_Decoder_TRANSFORMER_14791867367496.

The reference decoder is affine in the positions: each frame step is
    pos_{t+1} = pos_t @ M + (d_t[b] + g[b,j]),   M = I + W_pe @ W3  (3x3)
(with W_final = [W1; W2; W3] split along its 768 input rows), so the whole
60-step scan has a closed form

    out[b, j, :, t] = X[b, j, :] @ Q_t + r_t[b, :]

where X = initial_grid,
    Q_t = M^t + (W_pe @ W2) @ S_t,          S_t = sum_{k<t} M^k
    r_t[b] = h @ S_t + D_t[b],              D_t = sum_{s=1..t} d_s M^{t-s}
    d_t[b] = (emb_table[t] + z @ W_clip + b_clip) @ W1
    h      = b_pe @ (W2 + W3) + b_final

All of Q/r are tiny (3x3 / per-batch 3-vectors) and are computed on the host
in float64.  The device kernel is then a single affine map per point
([3 feats + bias] -> 180 outputs).

Precision budget: the harness gate is rel_err < 2e-2, so inputs are plain
bf16 (X and Q single-chunk; the large accumulated offset r split into two
bf16 chunks against two 1.0 bias rows) and the output is written as bf16
then upcast to f32 on gather.  Measured end-to-end error 1.7e-3.  K-stack
per tile is 5 rows, two tiles fused block-diagonally per matmul (K=10,
N=360).

The machine model this kernel is built around (all measured via NTFF):
PE streams one 128-wide psum column per cycle at 1.2 GHz regardless of
input dtype, so the 64 matmuls x 360 columns are a hard ~19.3 us floor;
DVE/ACT drain psum at 1 elem/lane/cycle (518/595 ns per [128,360] copy),
which exactly keeps pace with PE when alternated per matmul; each
dma_start's descriptors serialize on one of 16 queues (~200 GB/s alone,
~26 GB/s for DRAM->SBUF partition descriptors).  Hence:
 - per-matmul alternating DVE/ACT copies (never lets any engine block
   long enough to event-sleep),
 - input chunks spread over the SP, ACT and DVE rings so the first
   matmul's operands land as early as possible,
 - a dozen warm-up matmuls on garbage data keep PE busy until the input
   semaphore is already set (and give the p-state ramp a head start),
 - the first output group goes out in eighth/eighth/quarter/half pieces
   so the stream starts immediately, and the last group is split into
   four column-chunks on four queues to cut the ~3.6 us single-queue
   drain to ~1 us.

Sharding: data-parallel over batch -- each of the 8 cores handles 4
batches (16384 points = 128 point-tiles = 64 packed matmuls).
"""

import numpy as np

BS, NFRAMES, NJOINTS, NFEATS, LATENT, CLIP = 32, 60, 4096, 3, 256, 512
NCORES = 8
B_PER_CORE = BS // NCORES                  # 4
PTS = B_PER_CORE * NJOINTS                 # 16384 points per core
NTILES = PTS // 128                        # 128 point-tiles per core
GROUPS = 8                                 # output DMA groups
TPG = NTILES // GROUPS                     # 16 tiles per group
FC = NFEATS * NFRAMES                      # 180 output columns per point
KR = 5                                     # K rows per tile (3 feats + 2 bias)
PAIR = 2                                   # tiles fused per matmul
MM_PER_G = TPG // PAIR                     # 8 matmuls per group
N_WARM = 5                                 # PE warm-up matmuls


def _split2(a):
    """Split f32 array into two bf16 chunks whose sum reproduces ~16
    mantissa bits.  Returned as f32 arrays holding bf16-representable
    values."""
    import ml_dtypes
    bf = ml_dtypes.bfloat16
    a = np.asarray(a, np.float32)
    a0 = a.astype(bf).astype(np.float32)
    a1 = (a - a0).astype(bf).astype(np.float32)
    return a0, a1


def _precompute(z, W_pe, b_pe, W_clip, b_clip, emb_table, W_final, b_final):
    """Host-side f64 computation of the closed-form coefficients.

    Returns Q_all [3, 180] and r_all [32, 180], column layout c = f*60 + t
    (matching the [.., 3, 60] innermost layout of the output)."""
    f64 = np.float64
    W_pe64 = np.asarray(W_pe, f64)
    W_fin = np.asarray(W_final, f64)
    W1, W2, W3 = W_fin[:LATENT], W_fin[LATENT:2 * LATENT], W_fin[2 * LATENT:]
    M = np.eye(3) + W_pe64 @ W3
    Gm = W_pe64 @ W2
    b_pe64 = np.asarray(b_pe, f64)
    h = b_pe64 @ W2 + b_pe64 @ W3 + np.asarray(b_final, f64)
    z_proj = np.asarray(z, f64) @ np.asarray(W_clip, f64) + np.asarray(b_clip, f64)
    d = (np.asarray(emb_table, f64)[None, :, :] + z_proj[:, None, :]) @ W1  # [32,60,3]

    Q = np.zeros((NFRAMES, 3, 3))
    R = np.zeros((NFRAMES, BS, 3))
    Q[0] = np.eye(3)
    Mt = np.eye(3)
    S = np.zeros((3, 3))
    D = np.zeros((BS, 3))
    for t in range(1, NFRAMES):
        S = S + Mt
        Mt = Mt @ M
        D = D @ M + d[:, t, :]
        Q[t] = Mt + Gm @ S
        R[t] = h @ S + D
    Q_all = Q.transpose(1, 2, 0).reshape(3, FC)     # [k, f*60+t]
    r_all = R.transpose(1, 2, 0).reshape(BS, FC)    # [b, f*60+t]
    return Q_all.astype(np.float32), r_all.astype(np.float32)


N_PS = 8      # psum slots (one bank each; a group cycles all 8)
N_STAGE = 3   # stage buffers


def _copy_seq(j):
    """(engine, 1-based position of copy j within that engine's stream).

    Copies alternate DVE/ACT by matmul index so both engines share every
    group's copy wall."""
    return ("v" if j % 2 == 0 else "a"), j // 2 + 1


def _build_bass():
    import concourse.mybir as mybir
    from concourse import bacc
    from concourse.bass import ts

    f32 = mybir.dt.float32
    bf16 = mybir.dt.bfloat16
    nc = bacc.Bacc(None, target_bir_lowering=False)
    RW = B_PER_CORE * PAIR * FC            # 1440 rhs columns
    half = MM_PER_G * 128 // 2             # 512
    xt = nc.dram_tensor("xt", [PAIR * KR, NTILES // PAIR * 128], bf16,
                        kind="ExternalInput")
    # in0 = rhs + the first half of group 0's stationary data, so one DMA
    # (one issue, ~10 descriptors) gates the first matmul
    in0 = nc.dram_tensor("in0", [PAIR * KR, RW + half], bf16,
                         kind="ExternalInput")
    out = nc.dram_tensor("out", [PTS, FC], bf16, kind="ExternalOutput")
    out_v = out[:].rearrange("(g j w) c -> g j (w c)", g=GROUPS, j=128, w=TPG)
    out_v4 = out[:].rearrange("(g j w) c -> g j w c", g=GROUPS, j=128, w=TPG)

    from contextlib import ExitStack
    ctx = ExitStack()
    in0_sb = ctx.enter_context(
        nc.sbuf_tensor("in0_sb", [PAIR * KR, RW + half], bf16))
    rhs_sb = in0_sb  # rhs lives in cols [0, RW)
    xt_sb = [ctx.enter_context(
        nc.sbuf_tensor(f"xt_sb{g}", [PAIR * KR, MM_PER_G * 128], bf16))
        for g in range(GROUPS)]
    stage = [ctx.enter_context(
        nc.sbuf_tensor(f"stage{i}", [128, TPG * FC], bf16))
        for i in range(N_STAGE)]
    psum = [ctx.enter_context(
        nc.psum_tensor(f"psum{i}", [128, PAIR * FC], f32))
        for i in range(N_PS)]
    s_c0a = ctx.enter_context(nc.semaphore("s_c0a"))
    s_chunk = [ctx.enter_context(nc.semaphore(f"s_chunk{g}"))
               for g in range(GROUPS)]
    s_pe = ctx.enter_context(nc.semaphore("s_pe"))
    s_cpv = ctx.enter_context(nc.semaphore("s_cpv"))
    s_cpa = ctx.enter_context(nc.semaphore("s_cpa"))
    s_slot = [ctx.enter_context(nc.semaphore(f"s_slot{i}"))
              for i in range(N_STAGE)]

    # ---- input DMAs, spread over three rings so the first matmul's
    # operands (chunk0a + rhs) land in parallel ASAP and later group
    # chunks trickle in behind.  Each dma_start's descriptors serialize
    # on one hw queue, so parallelism comes from separate dma_starts. ----
    nc.sync.dma_start(out=in0_sb[:], in_=in0[:]).then_inc(s_c0a, 16)
    nc.sync.dma_start(out=xt_sb[0][:, half:],
                      in_=xt[:, half:MM_PER_G * 128]).then_inc(s_chunk[0], 16)
    ring = {1: nc.scalar, 2: nc.scalar, 3: nc.sync, 4: nc.gpsimd,
            5: nc.gpsimd, 6: nc.gpsimd, 7: nc.gpsimd}
    for g in range(1, GROUPS):
        ring[g].dma_start(
            out=xt_sb[g][:], in_=xt[:, ts(g, MM_PER_G * 128)]
        ).then_inc(s_chunk[g], 16)

    def stat_ap(g, sp):
        """Stationary [K, 128] slab for matmul sp of group g."""
        if g == 0 and sp < MM_PER_G // 2:
            return in0_sb[:, RW + 128 * sp:RW + 128 * (sp + 1)]
        return xt_sb[g][:, ts(sp, 128)]

    # out-DMA inc totals per stage slot, recorded in emission order so the
    # slot-reuse waits below match however many DMAs read the slot.
    dma_count = {0: 3, GROUPS - 2: 2, GROUPS - 1: 4}
    slot_reads_before = {}
    seen = [0] * N_STAGE
    for g in range(GROUPS):
        slot_reads_before[g] = seen[g % N_STAGE]
        seen[g % N_STAGE] += dma_count.get(g, 1)

    def copies(engine, s_cp_self, g, parity):
        st = stage[g % N_STAGE]
        for sp in range(parity, MM_PER_G, 2):
            j = g * MM_PER_G + sp
            if sp == parity and g >= N_STAGE:
                # stage slot reuse: wait for every out-DMA that read it
                engine.wait_ge(s_slot[g % N_STAGE],
                               16 * slot_reads_before[g])
            engine.wait_ge(s_pe, j + 1)
            if parity == 0:
                nc.vector.tensor_copy(
                    out=st[:, ts(sp, PAIR * FC)], in_=psum[j % N_PS][:]
                ).then_inc(s_cp_self, 1)
            else:
                nc.scalar.copy(
                    out=st[:, ts(sp, PAIR * FC)], in_=psum[j % N_PS][:]
                ).then_inc(s_cp_self, 1)

    for g in range(GROUPS):
        copies(nc.scalar, s_cpa, g, 1)

    # ---- DVE: even-slot copies ----
    for g in range(GROUPS):
        copies(nc.vector, s_cpv, g, 0)

    # ---- PE: warm-up matmuls on garbage data (keep the engine busy and
    # the clock ramping until the input semaphores are already set), then
    # the real matmuls ----
    for w in range(N_WARM):
        nc.tensor.matmul(
            psum[N_PS - 1][:],
            xt_sb[GROUPS - 1][:, ts(w % MM_PER_G, 128)],
            rhs_sb[:, :PAIR * FC],
            start=True, stop=True,
        )
    for g in range(GROUPS):
        lb = g // 2
        for sp in range(MM_PER_G):
            j = g * MM_PER_G + sp
            if g == 0:
                if sp == 0:
                    nc.tensor.wait_ge(s_c0a, 16)
                elif sp == MM_PER_G // 2:
                    nc.tensor.wait_ge(s_chunk[0], 16)
            elif sp == 0:
                nc.tensor.wait_ge(s_chunk[g], 16)
            if j >= N_PS:
                # psum slot reuse: wait for the copy that drained it
                eng, pos = _copy_seq(j - N_PS)
                nc.tensor.wait_ge(s_cpv if eng == "v" else s_cpa, pos)
            nc.tensor.matmul(
                psum[j % N_PS][:],
                stat_ap(g, sp),
                rhs_sb[:, ts(lb, PAIR * FC)],
                start=True, stop=True,
            ).then_inc(s_pe, 1)

    # ---- SP: output DMAs ----
    for g in range(GROUPS):
        st = stage[g % N_STAGE]
        if g == 0:
            # quarter/quarter/half DMAs: the stream starts right after the
            # first two copies land, with descriptor-efficient sizes
            for nv, na, w0, w1 in ((1, 1, 0, 4), (2, 2, 4, 8),
                                   (4, 4, 8, TPG)):
                nc.sync.wait_ge(s_cpv, nv)
                nc.sync.wait_ge(s_cpa, na)
                nc.sync.dma_start(
                    out=out_v4[0][:, w0:w1, :],
                    in_=st[:, w0 * FC:w1 * FC],
                ).then_inc(s_slot[0], 16)
            continue
        n_half = MM_PER_G * (g + 1) // 2
        if g == GROUPS - 2:
            # second-to-last group in halves to start its drain earlier
            for q in range(2):
                w0, w1 = q * TPG // 2, (q + 1) * TPG // 2
                n = MM_PER_G * g // 2 + 2 * (q + 1)
                nc.sync.wait_ge(s_cpv, n)
                nc.sync.wait_ge(s_cpa, n)
                nc.sync.dma_start(
                    out=out_v4[g][:, w0:w1, :],
                    in_=st[:, w0 * FC:w1 * FC],
                ).then_inc(s_slot[g % N_STAGE], 16)
            continue
        if g == GROUPS - 1:
            # last group: four quarter DMAs on four queues so the final
            # drain is ~4x shorter than one serialized 0.74 MB DMA
            for q in range(4):
                w0, w1 = q * TPG // 4, (q + 1) * TPG // 4
                nv = na = MM_PER_G * g // 2 + (q + 1)
                nc.sync.wait_ge(s_cpv, nv)
                nc.sync.wait_ge(s_cpa, na)
                nc.sync.dma_start(
                    out=out_v4[g][:, w0:w1, :],
                    in_=st[:, w0 * FC:w1 * FC],
                ).then_inc(s_slot[g % N_STAGE], 16)
            continue
        nc.sync.wait_ge(s_cpv, n_half)
        nc.sync.wait_ge(s_cpa, n_half)
        nc.sync.dma_start(out=out_v[g], in_=st[:]).then_inc(
            s_slot[g % N_STAGE], 16)

    ctx.close()
    nc.finalize()
    return nc


_NC_CACHE = None
_LAST_RESULTS = None  # BassKernelResults of the most recent run (for profiling)


def kernel(z, mask, initial_grid, W_pe, b_pe, W_clip, b_clip, emb_table,
           W_final, b_final):
    global _NC_CACHE, _LAST_RESULTS
    import ml_dtypes
    from concourse import bass_utils

    bf = ml_dtypes.bfloat16
    Q_all, r_all = _precompute(z, W_pe, b_pe, W_clip, b_clip, emb_table,
                               W_final, b_final)
    Q0 = Q_all.astype(bf).astype(np.float32)            # [3, 180]
    X = np.ascontiguousarray(np.asarray(initial_grid), dtype=np.float32)

    in_maps = []
    for c in range(NCORES):
        Xc = X[B_PER_CORE * c:B_PER_CORE * (c + 1)].reshape(PTS, NFEATS)
        # point p = g*2048 + j*16 + w lives at tile (g, w), psum partition j
        X4 = Xc.reshape(GROUPS, 128, TPG, NFEATS).transpose(3, 0, 2, 1)
        A = np.empty((GROUPS, TPG, KR, 128), np.float32)
        A[:, :, 0:NFEATS, :] = X4.transpose(1, 2, 0, 3)  # single bf16 chunk
        A[:, :, NFEATS:KR, :] = 1.0                      # bias rows (r0, r1)
        # matmul s covers tiles (2*(s%8), 2*(s%8)+1) of group s//8;
        # stationary rows KR*a.. hold tile a of the pair
        xt_host = (A.reshape(GROUPS, MM_PER_G, PAIR, KR, 128)
                   .transpose(2, 3, 0, 1, 4)
                   .reshape(PAIR * KR, NTILES // PAIR * 128)).astype(bf)

        rhs_host = np.zeros((PAIR * KR, B_PER_CORE * PAIR * FC), np.float32)
        for lb in range(B_PER_CORE):
            r0, r1 = _split2(r_all[B_PER_CORE * c + lb])  # 2 x [180]
            R = np.empty((KR, FC), np.float32)
            R[0:NFEATS] = Q0
            R[NFEATS] = r0
            R[NFEATS + 1] = r1
            for a in range(PAIR):                       # block-diagonal
                rhs_host[KR * a:KR * (a + 1),
                         lb * PAIR * FC + FC * a: lb * PAIR * FC + FC * (a + 1)] = R
        in0_host = np.concatenate(
            [rhs_host.astype(bf), xt_host[:, :MM_PER_G * 128 // 2]], axis=1)
        in_maps.append({"xt": np.ascontiguousarray(xt_host),
                        "in0": np.ascontiguousarray(in0_host)})

    if _NC_CACHE is None:
        _NC_CACHE = _build_bass()
    res = bass_utils.run_bass_kernel_spmd(
        _NC_CACHE, in_maps, core_ids=list(range(NCORES))
    )
    _LAST_RESULTS = res

    out = np.empty((BS, NJOINTS, NFEATS, NFRAMES), np.float32)
    for c in range(NCORES):
        out[B_PER_CORE * c:B_PER_CORE * (c + 1)] = (
            np.asarray(res.results[c]["out"]).astype(np.float32)
            .reshape(B_PER_CORE, NJOINTS, NFEATS, NFRAMES)
        )
    return out
